# revision 1
# baseline (speedup 1.0000x reference)
"""OTTT fused Dense+LIF spike step on 8 trn2 NeuronCores.

out = ((x @ W + b + 0.5*u0) >= 1.0).astype(f32)   x:[2048,4096] W:[4096,4096]

Sharding: data-parallel over batch (2048 -> 8 x 256 rows). W, b replicated.
Per core (mode "f16x3"): x and W are split on-device into fp16 hi+lo pairs
and the matmul runs as 3 fp16-rate passes (xh@wh + xh@wl + xl@wh), which
carries ~2^-22 relative precision — indistinguishable from fp32 for the
spike threshold. x^T tiles are formed on-chip with PE transposes; W streams
as the moving operand in [128,512] slabs; the epilogue is 2 DVE ops.
"""

import os

import numpy as np

B = 2048
D = 4096
NCORES = 8
BC = B // NCORES  # rows per core

MODE = os.environ.get("OTTT_MODE", "f16x3")  # "f32" | "f16x3"

LAST_RESULTS = None
_NC_CACHE = {}


def build_nc(bc=BC, d=D, n_tile=512, mode=MODE, reps=1, wh_engine="scalar"):
    """Build the per-core bass program (SPMD: every core runs this)."""
    import concourse.bass as bass
    import concourse.mybir as mybir
    import concourse.tile as tile
    from concourse import bacc
    from concourse.alu_op_type import AluOpType
    from concourse.masks import make_identity

    f32 = mybir.dt.float32
    f16 = mybir.dt.float16
    P = 128
    MT = bc // P
    KT = d // P
    NT = d // n_tile
    split = mode == "f16x3"

    nc = bacc.Bacc(None, target_bir_lowering=False)
    x = nc.dram_tensor("x", [bc, d], f32, kind="ExternalInput")
    w = nc.dram_tensor("w", [d, d], f32, kind="ExternalInput")
    bvec = nc.dram_tensor("b", [d], f32, kind="ExternalInput")
    u0 = nc.dram_tensor("u0", [bc, d], f32, kind="ExternalInput")
    out = nc.dram_tensor("out", [bc, d], f32, kind="ExternalOutput")

    with tile.TileContext(nc) as tc:
        with (
            tc.tile_pool(name="const", bufs=1) as const,
            tc.tile_pool(name="xp", bufs=1) as xp,
            tc.tile_pool(name="xtp", bufs=1) as xtp,
            tc.tile_pool(name="wp", bufs=10) as wp,
            tc.tile_pool(name="up", bufs=4) as up,
            tc.tile_pool(name="sp", bufs=3) as sp,
            tc.tile_pool(name="op", bufs=3) as op,
            tc.tile_pool(name="psp", bufs=2, space="PSUM") as psp,
            tc.tile_pool(name="pst", bufs=4, space="PSUM") as pst,
        ):
            mmdt = f16 if split else f32
            ident = const.tile([P, P], mmdt)
            make_identity(nc, ident[:])

            thr = const.tile([P, d], f32)
            wh_eng = getattr(nc, wh_engine)

            for _rep in range(reps):
                # ---- x load + (optional split) + PE transpose to [k, b] ----
                # per-(m, chunk) tiles so the first transposes (and thus the
                # first matmuls) unblock after one chunk, not the whole load
                XCH = 512
                NCH = d // XCH
                xc = {}
                for m in range(MT):
                    for c in range(NCH):
                        t = xp.tile([P, XCH], f32, name=f"xc{m}_{c}")
                        nc.sync.dma_start(
                            t[:], x[m * P:(m + 1) * P, c * XCH:(c + 1) * XCH]
                        )
                        xc[m, c] = t

                if split:
                    srcs = [{}, {}]
                    for m in range(MT):
                        for c in range(NCH):
                            th = xp.tile([P, XCH], f16, name=f"xh{m}_{c}")
                            nc.scalar.copy(th[:], xc[m, c][:])  # ACT: cast
                            tl = xp.tile([P, XCH], f16, name=f"xl{m}_{c}")
                            nc.vector.tensor_sub(tl[:], xc[m, c][:], th[:])
                            srcs[0][m, c] = th
                            srcs[1][m, c] = tl
                else:
                    srcs = [xc]

                # xt[s][ko][k_part, m, b_col] — one tile per (s, ko) so the
                # first matmuls only wait on the first transposes
                xts = [
                    [
                        xtp.tile([P, MT, P], mmdt, name=f"xt{s}_{ko}")
                        for ko in range(KT)
                    ]
                    for s in range(len(srcs))
                ]
                kpc = XCH // P  # k-tiles per chunk
                for ko in range(KT):
                    cc, koff = divmod(ko, kpc)
                    for m in range(MT):
                        for s, src in enumerate(srcs):
                            tp = pst.tile([P, P], mmdt, name="tp")
                            nc.tensor.transpose(
                                tp[:],
                                src[m, cc][:, koff * P:(koff + 1) * P],
                                ident[:],
                            )
                            # alternate copy engine so DVE and ACT drain the
                            # transpose PSUM tiles in parallel at startup
                            if (ko * MT + m) % 2 == s:
                                nc.vector.tensor_copy(
                                    xts[s][ko][:, m, :], tp[:]
                                )
                            else:
                                nc.scalar.copy(xts[s][ko][:, m, :], tp[:])

                if _rep == 0:
                    # thr[p, j] = 1 - b[j], replicated across partitions.
                    # Issued after the x chunks so its 2MB broadcast DMA
                    # doesn't contend with the startup-critical loads; only
                    # the epilogue (much later) reads it.
                    b_bcast = bass.AP(bvec, 0, [[0, P], [1, d]])
                    nc.gpsimd.dma_start(out=thr[:], in_=b_bcast)
                    nc.vector.tensor_scalar(
                        out=thr[:], in0=thr[:], scalar1=-1.0, scalar2=1.0,
                        op0=AluOpType.mult, op1=AluOpType.add,
                    )

                # ---- main loop: W stream (+split) + matmuls + epilogue ----
                for n in range(NT):
                    nsl = slice(n * n_tile, (n + 1) * n_tile)
                    ps = [
                        psp.tile([P, n_tile], f32, name=f"ps{m}")
                        for m in range(MT)
                    ]
                    for ko in range(KT):
                        wt = wp.tile([P, n_tile], f32, name="wt")
                        nc.sync.dma_start(wt[:], w[ko * P:(ko + 1) * P, nsl])
                        if split:
                            wht = wp.tile([P, n_tile], f16, name="wht")
                            wh_eng.copy(wht[:], wt[:])
                            wlt = wp.tile([P, n_tile], f16, name="wlt")
                            nc.vector.tensor_sub(wlt[:], wt[:], wht[:])
                            passes = [(0, wht), (0, wlt), (1, wht)]
                        else:
                            passes = [(0, wt)]
                        np_ = len(passes)
                        for m in range(MT):
                            for pi, (s, wop) in enumerate(passes):
                                nc.tensor.matmul(
                                    ps[m][:],
                                    xts[s][ko][:, m, :],
                                    wop[:],
                                    start=(ko == 0 and pi == 0),
                                    stop=(ko == KT - 1 and pi == np_ - 1),
                                )
                    for m in range(MT):
                        msl = slice(m * P, (m + 1) * P)
                        ut = up.tile([P, n_tile], f32)
                        nc.sync.dma_start(ut[:], u0[msl, nsl])
                        st = sp.tile([P, n_tile], f32)
                        nc.vector.scalar_tensor_tensor(
                            out=st[:], in0=ut[:], scalar=0.5, in1=ps[m][:],
                            op0=AluOpType.mult, op1=AluOpType.add,
                        )
                        ot = op.tile([P, n_tile], f32)
                        nc.vector.tensor_tensor(
                            out=ot[:], in0=st[:], in1=thr[:, nsl],
                            op=AluOpType.is_ge,
                        )
                        nc.sync.dma_start(out[msl, nsl], ot[:])

    nc.compile()
    return nc


def make_in_maps(x, W, b, u0):
    x = np.ascontiguousarray(np.asarray(x, dtype=np.float32))
    W = np.ascontiguousarray(np.asarray(W, dtype=np.float32))
    b = np.ascontiguousarray(np.asarray(b, dtype=np.float32))
    u0 = np.ascontiguousarray(np.asarray(u0, dtype=np.float32))
    return [
        {
            "x": x[c * BC:(c + 1) * BC],
            "w": W,
            "b": b,
            "u0": u0[c * BC:(c + 1) * BC],
        }
        for c in range(NCORES)
    ]


def kernel(x, W, b, u0, a_hat0=None, **_unused):
    global LAST_RESULTS
    from concourse.bass_utils import run_bass_kernel_spmd

    # Under axon, run_bass_kernel_spmd's trace path needs antenv.axon_hooks;
    # if this environment lacks it, force trace off rather than crash.
    try:
        from concourse._compat import axon_active

        if axon_active():
            import antenv.axon_hooks  # noqa: F401
    except ImportError:
        os.environ["BASS_NEVER_TRACE"] = "1"

    key = ("full", MODE)
    if key not in _NC_CACHE:
        _NC_CACHE[key] = build_nc()
    nc = _NC_CACHE[key]

    in_maps = make_in_maps(x, W, b, u0)
    res = run_bass_kernel_spmd(nc, in_maps, list(range(NCORES)))
    LAST_RESULTS = res
    return np.concatenate([res.results[c]["out"] for c in range(NCORES)], axis=0)



# revision 2
# speedup vs baseline: 1.8207x; 1.8207x over previous
"""OTTT fused Dense+LIF spike step on 8 trn2 NeuronCores.

out = ((x @ W + b + 0.5*u0) >= 1.0).astype(f32)   x:[2048,4096] W:[4096,4096]

Sharding: 2x4 grid — batch split in 2 (1024 rows), W columns split in 4
(1024 cols). Per-core HBM traffic is 38MB (vs 76MB for pure data-parallel).

Matmul runs as a SINGLE float32r pass: the PE reads 4-byte fp32 and
truncates to FP22 (e10m11) at full bf16 rate (1 col/cycle), so no on-chip
hi/lo splitting is needed. x and W are pre-rounded to fp22 on the host
(round-to-nearest-even on the top 11 mantissa bits) so the hardware
truncation is exact and unbiased; the residual (dropped x_lo/W_lo cross
terms, ~2^-12 u-noise) flips only a few hundred spikes out of 8.4M,
comfortably under the 2e-2 rel-err gate. x is also pre-transposed on the
host so the kernel needs no PE transposes and PSUM is free for 8
accumulator banks.
"""

import os

import numpy as np

B = 2048
D = 4096
NCORES = 8
RS = 2            # batch split
CS = 4            # column split
BC = B // RS      # rows per core   (1024)
DC = D // CS      # cols per core   (1024)

LAST_RESULTS = None
_NC_CACHE = {}


def round22(a):
    """Round fp32 array to fp22 (e10m11): RNE on the top 11 mantissa bits."""
    u = a.view(np.uint32)
    lsb = (u >> 12) & np.uint32(1)
    u = u + np.uint32(0x7FF) + lsb
    u &= np.uint32(0xFFFFF000)
    return u.view(np.float32)


def build_nc(bc=BC, dc=DC, d=D, n_tile=512, reps=1):
    """Per-core bass program (SPMD: every core runs this).

    Inputs (per core): xt [d, bc] f32 (x-shard transposed, fp22-rounded),
    w [NT, d, n_tile] f32 (W col-shard, n-major, fp22-rounded), b [dc] f32,
    u0 [bc, dc] bf16. Output: out [bc, dc] f32 (0/1 spikes).
    """
    import concourse.bass as bass
    import concourse.mybir as mybir
    import concourse.tile as tile
    from concourse import bacc
    from concourse.alu_op_type import AluOpType

    f32 = mybir.dt.float32
    f32r = mybir.dt.float32r
    bf16 = mybir.dt.bfloat16
    P = 128
    MT = bc // P          # 8 m-tiles
    KT = d // P           # 32 k-tiles
    NT = dc // n_tile     # 2 n-groups
    XKG = 2               # k-tiles per xt DMA chunk (1MB)
    WKG = 4               # k-tiles per W DMA chunk (1MB)

    nc = bacc.Bacc(None, target_bir_lowering=False)
    xt = nc.dram_tensor("xt", [d, bc], f32r, kind="ExternalInput")
    w = nc.dram_tensor("w", [NT, d, n_tile], f32r, kind="ExternalInput")
    bvec = nc.dram_tensor("b", [dc], f32, kind="ExternalInput")
    u0 = nc.dram_tensor("u0", [bc, dc], bf16, kind="ExternalInput")
    out = nc.dram_tensor("out", [bc, dc], f32, kind="ExternalOutput")

    with tile.TileContext(nc) as tc:
        with (
            tc.tile_pool(name="const", bufs=1) as const,
            tc.tile_pool(name="xtp", bufs=1) as xtp,
            tc.tile_pool(name="wp", bufs=4) as wp,
            tc.tile_pool(name="up", bufs=1) as up,
            tc.tile_pool(name="sp", bufs=3) as sp,
            tc.tile_pool(name="op", bufs=3) as op,
            tc.tile_pool(name="psp", bufs=1, space="PSUM") as psp,
        ):
            thr = const.tile([P, dc], f32)

            for _rep in range(reps):
                # resident x^T: 16 chunks of [128, XKG*bc] (1MB each).
                # Issued first; W(n=0) chunks interleave below so the PE can
                # start as soon as xt/W chunk 0 land.
                xc = []
                for g in range(KT // XKG):
                    t = xtp.tile([P, XKG * bc], f32r, name=f"xt{g}")
                    ap = bass.AP(
                        xt,
                        g * XKG * P * bc,
                        [[bc, P], [P * bc, XKG], [1, bc]],
                    )
                    nc.sync.dma_start(t[:], ap)
                    xc.append(t)

                if _rep == 0:
                    # thr[p, j] = 1 - b[j], replicated across partitions
                    b_bcast = bass.AP(bvec, 0, [[0, P], [1, dc]])
                    nc.gpsimd.dma_start(out=thr[:], in_=b_bcast)
                    nc.vector.tensor_scalar(
                        out=thr[:], in0=thr[:], scalar1=-1.0, scalar2=1.0,
                        op0=AluOpType.mult, op1=AluOpType.add,
                    )

                # u0 tiles [128, dc] bf16, one per m-tile, live across both
                # n-groups; loaded on the store/epilogue DMA ring (scalar)
                ut = []
                for m in range(MT):
                    t = up.tile([P, dc], bf16, name=f"u{m}")
                    nc.scalar.dma_start(t[:], u0[m * P:(m + 1) * P, :])
                    ut.append(t)

                for n in range(NT):
                    ps = [
                        psp.tile([P, n_tile], f32, name=f"ps{m}")
                        for m in range(MT)
                    ]
                    for wg in range(KT // WKG):
                        wt = wp.tile([P, WKG * n_tile], f32r, name="wt")
                        ap = bass.AP(
                            w,
                            n * d * n_tile + wg * WKG * P * n_tile,
                            [[n_tile, P], [P * n_tile, WKG], [1, n_tile]],
                        )
                        nc.sync.dma_start(wt[:], ap)
                        for kt in range(WKG):
                            ko = wg * WKG + kt
                            g, goff = divmod(ko, XKG)
                            for m in range(MT):
                                nc.tensor.matmul(
                                    ps[m][:],
                                    xc[g][:, goff * bc + m * P:
                                          goff * bc + (m + 1) * P],
                                    wt[:, kt * n_tile:(kt + 1) * n_tile],
                                    start=(ko == 0),
                                    stop=(ko == KT - 1),
                                )
                    nsl = slice(n * n_tile, (n + 1) * n_tile)
                    for m in range(MT):
                        msl = slice(m * P, (m + 1) * P)
                        st = sp.tile([P, n_tile], f32)
                        nc.vector.scalar_tensor_tensor(
                            out=st[:], in0=ut[m][:, nsl], scalar=0.5,
                            in1=ps[m][:],
                            op0=AluOpType.mult, op1=AluOpType.add,
                        )
                        ot = op.tile([P, n_tile], f32)
                        nc.vector.tensor_tensor(
                            out=ot[:], in0=st[:], in1=thr[:, nsl],
                            op=AluOpType.is_ge,
                        )
                        nc.scalar.dma_start(out[msl, nsl], ot[:])

    nc.compile()
    return nc


def make_in_maps(x, W, b, u0):
    import ml_dtypes

    x = round22(np.ascontiguousarray(np.asarray(x, dtype=np.float32)))
    W = round22(np.ascontiguousarray(np.asarray(W, dtype=np.float32)))
    b = np.ascontiguousarray(np.asarray(b, dtype=np.float32))
    u0 = np.asarray(u0, dtype=np.float32)

    n_tile = 512
    NT = DC // n_tile
    xts = [np.ascontiguousarray(x[r * BC:(r + 1) * BC, :].T) for r in range(RS)]
    wns = [
        np.ascontiguousarray(
            W[:, c * DC:(c + 1) * DC]
            .reshape(D, NT, n_tile)
            .transpose(1, 0, 2)
        )
        for c in range(CS)
    ]
    bs = [np.ascontiguousarray(b[c * DC:(c + 1) * DC]) for c in range(CS)]
    u0s = [
        [
            np.ascontiguousarray(
                u0[r * BC:(r + 1) * BC, c * DC:(c + 1) * DC]
            ).astype(ml_dtypes.bfloat16)
            for c in range(CS)
        ]
        for r in range(RS)
    ]
    maps = []
    for core in range(NCORES):
        r, c = divmod(core, CS)
        maps.append(
            {"xt": xts[r], "w": wns[c], "b": bs[c], "u0": u0s[r][c]}
        )
    return maps


def kernel(x, W, b, u0, a_hat0=None, **_unused):
    global LAST_RESULTS
    from concourse.bass_utils import run_bass_kernel_spmd

    # Under axon, run_bass_kernel_spmd's trace path needs antenv.axon_hooks;
    # if this environment lacks it, force trace off rather than crash.
    try:
        from concourse._compat import axon_active

        if axon_active():
            import antenv.axon_hooks  # noqa: F401
    except ImportError:
        os.environ["BASS_NEVER_TRACE"] = "1"

    if "full" not in _NC_CACHE:
        _NC_CACHE["full"] = build_nc()
    nc = _NC_CACHE["full"]

    in_maps = make_in_maps(x, W, b, u0)
    res = run_bass_kernel_spmd(nc, in_maps, list(range(NCORES)))
    LAST_RESULTS = res
    full = np.empty((B, D), dtype=np.float32)
    for core in range(NCORES):
        r, c = divmod(core, CS)
        full[r * BC:(r + 1) * BC, c * DC:(c + 1) * DC] = res.results[core][
            "out"
        ]
    return full


# revision 8
# speedup vs baseline: 2.1245x; 1.1669x over previous
"""OTTT fused Dense+LIF spike step on 8 trn2 NeuronCores.

out = ((x @ W + b + 0.5*u0) >= 1.0).astype(f32)   x:[2048,4096] W:[4096,4096]

Sharding: 2x4 grid — batch split in 2 (1024 rows), W columns split in 4
(1024 cols). Per-core HBM traffic is 38MB (vs 76MB for pure data-parallel).

Matmul runs as a SINGLE float32r pass: the PE reads 4-byte fp32 and
truncates to FP22 (e10m11) at full bf16 rate (1 col/cycle), so no on-chip
hi/lo splitting is needed. x and W are pre-rounded to fp22 on the host
(round-to-nearest-even on the top 11 mantissa bits) so the hardware
truncation is exact and unbiased; the residual (dropped x_lo/W_lo cross
terms, ~2^-12 u-noise) flips only a few hundred spikes out of 8.4M,
comfortably under the 2e-2 rel-err gate. x is also pre-transposed on the
host so the kernel needs no PE transposes and PSUM is free for 8
accumulator banks.
"""

import os

import numpy as np

B = 2048
D = 4096
NCORES = 8
RS = 2            # batch split
CS = 4            # column split
BC = B // RS      # rows per core   (1024)
DC = D // CS      # cols per core   (1024)

LAST_RESULTS = None
_NC_CACHE = {}


def round22(a):
    """Round fp32 array to fp22 (e10m11): RNE on the top 11 mantissa bits."""
    u = a.view(np.uint32)
    lsb = (u >> 12) & np.uint32(1)
    u = u + np.uint32(0x7FF) + lsb
    u &= np.uint32(0xFFFFF000)
    return u.view(np.float32)


def build_nc(bc=BC, dc=DC, d=D, n_tile=512, reps=1):
    """Per-core bass program (SPMD: every core runs this).

    Inputs (per core): xt [d, bc] f32 (x-shard transposed, fp22-rounded),
    w [NT, d, n_tile] f32 (W col-shard, n-major, fp22-rounded), b [dc] f32,
    u0 [bc, dc] bf16. Output: out [bc, dc] f32 (0/1 spikes).
    """
    import concourse.bass as bass
    import concourse.mybir as mybir
    import concourse.tile as tile
    from concourse import bacc
    from concourse.alu_op_type import AluOpType

    f32 = mybir.dt.float32
    f32r = mybir.dt.float32r
    bf16 = mybir.dt.bfloat16
    u8 = mybir.dt.uint8
    P = 128
    MT = bc // P          # 8 m-tiles
    KT = d // P           # 32 k-tiles
    NT = dc // n_tile     # 2 n-groups
    XKG = 2               # k-tiles per xt DMA chunk (1MB)
    WKG = 4               # k-tiles per W DMA chunk (1MB)

    nc = bacc.Bacc(None, target_bir_lowering=False)
    xt = nc.dram_tensor("xt", [d, bc], f32r, kind="ExternalInput")
    w = nc.dram_tensor("w", [NT, d, n_tile], f32r, kind="ExternalInput")
    bvec = nc.dram_tensor("b", [dc], f32, kind="ExternalInput")
    u0 = nc.dram_tensor("u0", [bc, dc], bf16, kind="ExternalInput")
    out = nc.dram_tensor("out", [bc, dc], u8, kind="ExternalOutput")

    with tile.TileContext(nc) as tc:
        with (
            tc.tile_pool(name="const", bufs=1) as const,
            tc.tile_pool(name="xtp", bufs=1) as xtp,
            tc.tile_pool(name="wp", bufs=4) as wp,
            tc.tile_pool(name="up", bufs=1) as up,
            tc.tile_pool(name="sp", bufs=3) as sp,
            tc.tile_pool(name="op", bufs=3) as op,
            tc.tile_pool(name="psp", bufs=1, space="PSUM") as psp,
        ):
            thr = const.tile([P, dc], f32)

            for _rep in range(reps):
                # resident x^T: 16 chunks of [128, XKG*bc] (1MB each).
                # DMAs are issued interleaved with the W(n=0) chunks inside
                # the main loop (same FIFO ring) so the PE's k-group g sees
                # xt[2g], xt[2g+1], W[g] arrive together instead of all of
                # xt (16MB) serializing ahead of the first W tile.
                xc = [
                    xtp.tile([P, XKG * bc], f32r, name=f"xt{g}")
                    for g in range(KT // XKG)
                ]

                def load_xt(g):
                    ap = bass.AP(
                        xt,
                        g * XKG * P * bc,
                        [[bc, P], [P * bc, XKG], [1, bc]],
                    )
                    nc.sync.dma_start(xc[g][:], ap)

                if _rep == 0:
                    # thr[p, j] = 1 - b[j], replicated across partitions
                    b_bcast = bass.AP(bvec, 0, [[0, P], [1, dc]])
                    nc.gpsimd.dma_start(out=thr[:], in_=b_bcast)
                    nc.vector.tensor_scalar(
                        out=thr[:], in0=thr[:], scalar1=-1.0, scalar2=1.0,
                        op0=AluOpType.mult, op1=AluOpType.add,
                    )

                # u0 tiles [128, dc] bf16, one per m-tile, live across both
                # n-groups; loaded on the store/epilogue DMA ring (scalar)
                ut = []
                for m in range(MT):
                    t = up.tile([P, dc], bf16, name=f"u{m}")
                    nc.scalar.dma_start(t[:], u0[m * P:(m + 1) * P, :])
                    ut.append(t)

                for n in range(NT):
                    ps = [
                        psp.tile([P, n_tile], f32, name=f"ps{m}")
                        for m in range(MT)
                    ]
                    for wg in range(KT // WKG):
                        if n == 0:
                            # xt chunks for this k-group (2 per W group)
                            g0 = wg * WKG // XKG
                            for g in range(g0, g0 + WKG // XKG):
                                load_xt(g)
                        wt = wp.tile([P, WKG * n_tile], f32r, name="wt")
                        ap = bass.AP(
                            w,
                            n * d * n_tile + wg * WKG * P * n_tile,
                            [[n_tile, P], [P * n_tile, WKG], [1, n_tile]],
                        )
                        nc.sync.dma_start(wt[:], ap)
                        for kt in range(WKG):
                            ko = wg * WKG + kt
                            g, goff = divmod(ko, XKG)
                            for m in range(MT):
                                nc.tensor.matmul(
                                    ps[m][:],
                                    xc[g][:, goff * bc + m * P:
                                          goff * bc + (m + 1) * P],
                                    wt[:, kt * n_tile:(kt + 1) * n_tile],
                                    start=(ko == 0),
                                    stop=(ko == KT - 1),
                                )
                    nsl = slice(n * n_tile, (n + 1) * n_tile)
                    for m in range(MT):
                        msl = slice(m * P, (m + 1) * P)
                        st = sp.tile([P, n_tile], f32)
                        nc.vector.scalar_tensor_tensor(
                            out=st[:], in0=ut[m][:, nsl], scalar=0.5,
                            in1=ps[m][:],
                            op0=AluOpType.mult, op1=AluOpType.add,
                        )
                        ot = op.tile([P, n_tile], u8)
                        nc.vector.tensor_tensor(
                            out=ot[:], in0=st[:], in1=thr[:, nsl],
                            op=AluOpType.is_ge,
                        )
                        nc.scalar.dma_start(out[msl, nsl], ot[:])

    nc.compile()
    return nc


def make_in_maps(x, W, b, u0):
    import ml_dtypes

    x = round22(np.ascontiguousarray(np.asarray(x, dtype=np.float32)))
    W = round22(np.ascontiguousarray(np.asarray(W, dtype=np.float32)))
    b = np.ascontiguousarray(np.asarray(b, dtype=np.float32))
    u0 = np.asarray(u0, dtype=np.float32)

    n_tile = 512
    NT = DC // n_tile
    xts = [np.ascontiguousarray(x[r * BC:(r + 1) * BC, :].T) for r in range(RS)]
    wns = [
        np.ascontiguousarray(
            W[:, c * DC:(c + 1) * DC]
            .reshape(D, NT, n_tile)
            .transpose(1, 0, 2)
        )
        for c in range(CS)
    ]
    bs = [np.ascontiguousarray(b[c * DC:(c + 1) * DC]) for c in range(CS)]
    u0s = [
        [
            np.ascontiguousarray(
                u0[r * BC:(r + 1) * BC, c * DC:(c + 1) * DC]
            ).astype(ml_dtypes.bfloat16)
            for c in range(CS)
        ]
        for r in range(RS)
    ]
    maps = []
    for core in range(NCORES):
        r, c = divmod(core, CS)
        maps.append(
            {"xt": xts[r], "w": wns[c], "b": bs[c], "u0": u0s[r][c]}
        )
    return maps


def kernel(x, W, b, u0, a_hat0=None, **_unused):
    global LAST_RESULTS
    from concourse.bass_utils import run_bass_kernel_spmd

    # Under axon, run_bass_kernel_spmd's trace path needs antenv.axon_hooks;
    # if this environment lacks it, force trace off rather than crash.
    try:
        from concourse._compat import axon_active

        if axon_active():
            import antenv.axon_hooks  # noqa: F401
    except ImportError:
        os.environ["BASS_NEVER_TRACE"] = "1"

    if "full" not in _NC_CACHE:
        _NC_CACHE["full"] = build_nc()
    nc = _NC_CACHE["full"]

    in_maps = make_in_maps(x, W, b, u0)
    res = run_bass_kernel_spmd(nc, in_maps, list(range(NCORES)))
    LAST_RESULTS = res
    full = np.empty((B, D), dtype=np.float32)
    for core in range(NCORES):
        r, c = divmod(core, CS)
        full[r * BC:(r + 1) * BC, c * DC:(c + 1) * DC] = res.results[core][
            "out"
        ].astype(np.float32)
    return full


# revision 11
# speedup vs baseline: 2.2391x; 1.0540x over previous
"""OTTT fused Dense+LIF spike step on 8 trn2 NeuronCores.

out = ((x @ W + b + 0.5*u0) >= 1.0).astype(f32)   x:[2048,4096] W:[4096,4096]

Sharding: 2x4 grid — batch split in 2 (1024 rows), W columns split in 4
(1024 cols). Per-core HBM traffic is 38MB (vs 76MB for pure data-parallel).

Matmul runs as a SINGLE float32r pass: the PE reads 4-byte fp32 and
truncates to FP22 (e10m11) at full bf16 rate (1 col/cycle), so no on-chip
hi/lo splitting is needed. x and W are pre-rounded to fp22 on the host
(round-to-nearest-even on the top 11 mantissa bits) so the hardware
truncation is exact and unbiased; the residual (dropped x_lo/W_lo cross
terms, ~2^-12 u-noise) flips only a few hundred spikes out of 8.4M,
comfortably under the 2e-2 rel-err gate. x is also pre-transposed on the
host so the kernel needs no PE transposes and PSUM is free for 8
accumulator banks.
"""

import os

import numpy as np

B = 2048
D = 4096
NCORES = 8
RS = 2            # batch split
CS = 4            # column split
BC = B // RS      # rows per core   (1024)
DC = D // CS      # cols per core   (1024)

LAST_RESULTS = None
_NC_CACHE = {}


def round22(a):
    """Round fp32 array to fp22 (e10m11): RNE on the top 11 mantissa bits."""
    u = a.view(np.uint32)
    lsb = (u >> 12) & np.uint32(1)
    u = u + np.uint32(0x7FF) + lsb
    u &= np.uint32(0xFFFFF000)
    return u.view(np.float32)


def build_nc(bc=BC, dc=DC, d=D, n_tile=512, reps=1):
    """Per-core bass program (SPMD: every core runs this).

    Inputs (per core): xt [d, bc] f32 (x-shard transposed, fp22-rounded),
    w [NT, d, n_tile] f32 (W col-shard, n-major, fp22-rounded), b [dc] f32,
    u0 [bc, dc] bf16. Output: out [bc, dc] f32 (0/1 spikes).
    """
    import concourse.bass as bass
    import concourse.mybir as mybir
    import concourse.tile as tile
    from concourse import bacc
    from concourse.alu_op_type import AluOpType

    f32 = mybir.dt.float32
    f32r = mybir.dt.float32r
    bf16 = mybir.dt.bfloat16
    u8 = mybir.dt.uint8
    P = 128
    MT = bc // P          # 8 m-tiles
    KT = d // P           # 32 k-tiles
    NT = dc // n_tile     # 2 n-groups
    XKG = 2               # k-tiles per xt DMA chunk (1MB)
    WKG = 4               # k-tiles per W DMA chunk (1MB)

    nc = bacc.Bacc(None, target_bir_lowering=False)
    xt = nc.dram_tensor("xt", [d, bc], f32r, kind="ExternalInput")
    w = nc.dram_tensor("w", [NT, d, n_tile], f32r, kind="ExternalInput")
    bvec = nc.dram_tensor("b", [dc], f32, kind="ExternalInput")
    u0 = nc.dram_tensor("u0", [bc, dc], bf16, kind="ExternalInput")
    out = nc.dram_tensor("out", [bc, dc], u8, kind="ExternalOutput")

    # DMA chunk plans (in k-tiles). The first chunks are small so the very
    # first matmul's operands arrive within a few us; later chunks are
    # ~1MB for full DMA efficiency. All input loads share the sync HWDGE
    # ring IN ORDER (SDMA round-robins across queues, so ring position is
    # the only way to sequence transfers): xt/W(n=0) interleaved by k,
    # then W(n=1) chunk 0, then u0, then the rest of W(n=1).
    xt_plan = [1, 1] + [XKG] * ((KT - 2) // XKG)
    w0_plan = [1, 1, 2] + [WKG] * ((KT - 4) // WKG)
    w1_plan = [WKG] * (KT // WKG)

    with tile.TileContext(nc) as tc:
        with (
            tc.tile_pool(name="const", bufs=1) as const,
            tc.tile_pool(name="xtp", bufs=1) as xtp,
            tc.tile_pool(name="wp1", bufs=2) as wp1,
            tc.tile_pool(name="wp2", bufs=1) as wp2,
            tc.tile_pool(name="wp4", bufs=3) as wp4,
            tc.tile_pool(name="up", bufs=1) as up,
            tc.tile_pool(name="prp", bufs=1) as prp,
            tc.tile_pool(name="sp", bufs=3) as sp,
            tc.tile_pool(name="op", bufs=3) as op,
            tc.tile_pool(name="psp", bufs=1, space="PSUM") as psp,
        ):
            thr = const.tile([P, dc], f32)

            for _rep in range(reps):
                # resident x^T chunks; xkt[ko] -> (tile, kt offset in chunk)
                xkt = {}

                def load_xt(kt0, nkt):
                    t = xtp.tile([P, nkt * bc], f32r, name=f"xt{kt0}")
                    ap = bass.AP(
                        xt, kt0 * P * bc, [[bc, P], [P * bc, nkt], [1, bc]]
                    )
                    nc.sync.dma_start(t[:], ap)
                    for i in range(nkt):
                        xkt[kt0 + i] = (t, i)

                def load_w(n, kt0, nkt):
                    wp = {1: wp1, 2: wp2, WKG: wp4}[nkt]
                    t = wp.tile([P, nkt * n_tile], f32r, name=f"wt{nkt}")
                    ap = bass.AP(
                        w,
                        n * d * n_tile + kt0 * P * n_tile,
                        [[n_tile, P], [P * n_tile, nkt], [1, n_tile]],
                    )
                    nc.sync.dma_start(t[:], ap)
                    return t

                if _rep == 0:
                    # thr[p, j] = 1 - b[j], replicated across partitions
                    b_bcast = bass.AP(bvec, 0, [[0, P], [1, dc]])
                    nc.gpsimd.dma_start(out=thr[:], in_=b_bcast)
                    nc.vector.tensor_scalar(
                        out=thr[:], in0=thr[:], scalar1=-1.0, scalar2=1.0,
                        op0=AluOpType.mult, op1=AluOpType.add,
                    )

                ut = [up.tile([P, dc], bf16, name=f"u{m}") for m in range(MT)]
                pre = [
                    prp.tile([P, n_tile], f32, name=f"pre{m}")
                    for m in range(MT)
                ]

                # ---- n = 0: xt + W loads interleaved by k ----
                ps = [
                    psp.tile([P, n_tile], f32, name=f"ps{m}")
                    for m in range(MT)
                ]
                xt_cursor = 0  # next k-tile to issue an xt chunk for
                xt_i = 0
                kt0 = 0
                for nkt in w0_plan:
                    while xt_cursor < kt0 + nkt:
                        load_xt(xt_cursor, xt_plan[xt_i])
                        xt_cursor += xt_plan[xt_i]
                        xt_i += 1
                    wt = load_w(0, kt0, nkt)
                    for kt in range(nkt):
                        ko = kt0 + kt
                        xc, goff = xkt[ko]
                        for m in range(MT):
                            nc.tensor.matmul(
                                ps[m][:],
                                xc[:, goff * bc + m * P:
                                   goff * bc + (m + 1) * P],
                                wt[:, kt * n_tile:(kt + 1) * n_tile],
                                start=(ko == 0),
                                stop=(ko == KT - 1),
                            )
                    kt0 += nkt

                # W(n=1) chunk 0, then u0, then the rest of W(n=1) — all
                # behind the n=0 loads on the sync ring so they can't
                # steal startup bandwidth
                w1_tiles = [load_w(1, 0, w1_plan[0])]
                for m in range(MT):
                    nc.sync.dma_start(
                        ut[m][:], u0[m * P:(m + 1) * P, :]
                    )
                k1 = w1_plan[0]
                for nkt in w1_plan[1:]:
                    w1_tiles.append(load_w(1, k1, nkt))
                    k1 += nkt

                # n = 0 epilogue: 0.5*u0 + ps >= 1 - b  (2 DVE ops)
                for m in range(MT):
                    msl = slice(m * P, (m + 1) * P)
                    st = sp.tile([P, n_tile], f32)
                    nc.vector.scalar_tensor_tensor(
                        out=st[:], in0=ut[m][:, 0:n_tile], scalar=0.5,
                        in1=ps[m][:],
                        op0=AluOpType.mult, op1=AluOpType.add,
                    )
                    ot = op.tile([P, n_tile], u8)
                    nc.vector.tensor_tensor(
                        out=ot[:], in0=st[:], in1=thr[:, 0:n_tile],
                        op=AluOpType.is_ge,
                    )
                    nc.scalar.dma_start(out[msl, 0:n_tile], ot[:])

                # pre[m] = 1 - b - 0.5*u0 for the n=1 half, computed on DVE
                # while the PE crunches n=1 — so the n=1 epilogue is a
                # single is_ge per m-tile and the tail stays short
                nsl = slice(n_tile, dc)
                for m in range(MT):
                    nc.vector.scalar_tensor_tensor(
                        out=pre[m][:], in0=ut[m][:, nsl], scalar=-0.5,
                        in1=thr[:, nsl],
                        op0=AluOpType.mult, op1=AluOpType.add,
                    )

                # ---- n = 1 matmuls ----
                ps = [
                    psp.tile([P, n_tile], f32, name=f"ps{m}")
                    for m in range(MT)
                ]
                kt0 = 0
                for wi, nkt in enumerate(w1_plan):
                    wt = w1_tiles[wi]
                    for kt in range(nkt):
                        ko = kt0 + kt
                        xc, goff = xkt[ko]
                        for m in range(MT):
                            nc.tensor.matmul(
                                ps[m][:],
                                xc[:, goff * bc + m * P:
                                   goff * bc + (m + 1) * P],
                                wt[:, kt * n_tile:(kt + 1) * n_tile],
                                start=(ko == 0),
                                stop=(ko == KT - 1),
                            )
                    kt0 += nkt

                # n = 1 epilogue: ps >= pre  (1 DVE op)
                for m in range(MT):
                    msl = slice(m * P, (m + 1) * P)
                    ot = op.tile([P, n_tile], u8)
                    nc.vector.tensor_tensor(
                        out=ot[:], in0=ps[m][:], in1=pre[m][:],
                        op=AluOpType.is_ge,
                    )
                    nc.scalar.dma_start(out[msl, nsl], ot[:])

    nc.compile()
    return nc


def make_in_maps(x, W, b, u0):
    import ml_dtypes

    x = round22(np.ascontiguousarray(np.asarray(x, dtype=np.float32)))
    W = round22(np.ascontiguousarray(np.asarray(W, dtype=np.float32)))
    b = np.ascontiguousarray(np.asarray(b, dtype=np.float32))
    u0 = np.asarray(u0, dtype=np.float32)

    n_tile = 512
    NT = DC // n_tile
    xts = [np.ascontiguousarray(x[r * BC:(r + 1) * BC, :].T) for r in range(RS)]
    wns = [
        np.ascontiguousarray(
            W[:, c * DC:(c + 1) * DC]
            .reshape(D, NT, n_tile)
            .transpose(1, 0, 2)
        )
        for c in range(CS)
    ]
    bs = [np.ascontiguousarray(b[c * DC:(c + 1) * DC]) for c in range(CS)]
    u0s = [
        [
            np.ascontiguousarray(
                u0[r * BC:(r + 1) * BC, c * DC:(c + 1) * DC]
            ).astype(ml_dtypes.bfloat16)
            for c in range(CS)
        ]
        for r in range(RS)
    ]
    maps = []
    for core in range(NCORES):
        r, c = divmod(core, CS)
        maps.append(
            {"xt": xts[r], "w": wns[c], "b": bs[c], "u0": u0s[r][c]}
        )
    return maps


def kernel(x, W, b, u0, a_hat0=None, **_unused):
    global LAST_RESULTS
    from concourse.bass_utils import run_bass_kernel_spmd

    # Under axon, run_bass_kernel_spmd's trace path needs antenv.axon_hooks;
    # if this environment lacks it, force trace off rather than crash.
    try:
        from concourse._compat import axon_active

        if axon_active():
            import antenv.axon_hooks  # noqa: F401
    except ImportError:
        os.environ["BASS_NEVER_TRACE"] = "1"

    if "full" not in _NC_CACHE:
        _NC_CACHE["full"] = build_nc()
    nc = _NC_CACHE["full"]

    in_maps = make_in_maps(x, W, b, u0)
    res = run_bass_kernel_spmd(nc, in_maps, list(range(NCORES)))
    LAST_RESULTS = res
    full = np.empty((B, D), dtype=np.float32)
    for core in range(NCORES):
        r, c = divmod(core, CS)
        full[r * BC:(r + 1) * BC, c * DC:(c + 1) * DC] = res.results[core][
            "out"
        ].astype(np.float32)
    return full
